# revision 13
# baseline (speedup 1.0000x reference)
"""Trainium2 Bass kernel for the sum-product "knowledge layer" network.

Computation (see problem reference):
  h0 = encode(x): 8194-row table [-inf, 0, pos0, neg0, pos1, neg1, ...]
       with pos = x (log-probs), neg = log(1 - exp(x)), per batch column.
  4 alternating layers, each: gather rows by ptrs, then segment-reduce over
  contiguous fanin groups (fanin 4 sum-of-logs "product" layers, fanin 2
  logsumexp "sum" layers).

Strategy (pure batch data-parallelism, 8 NeuronCores):
  - Shard the 512 batch columns 8 ways -> 64 columns per core.
  - Per core every tensor lives in DRAM as [rows, 64] fp32; one row = 256B.
  - Gathers use the SWDGE dma_gather instruction: int16 index list in SBUF,
    each index pulls one 256B row from the DRAM table; index list position j
    lands at SBUF partition j%128, free slot j//128.
  - Host pre-permutes each layer's ptrs so that the edges of output group g
    land on partition g//C (C = n_out/128) at free slots fanin*(g%C)+k.
    Segment reduction then becomes strided free-dim vector ops, and the
    layer output [128, C, 64] DMAs back to DRAM in natural row order
    (partition p holds rows p*C .. p*C+C-1, fully contiguous per partition).
  - Sum layers: logsumexp(a,b) = max + softplus(min - max) on DVE + ACT.
"""

import numpy as np

P = 128
B = 64  # batch columns per core
NCORES = 8
N_VARS = 4096
BATCH = 512
TAB0 = 2 * N_VARS + 2  # 8194
OUT_SIZES = [16384, 8192, 4096, 2048]
FANINS = [4, 2, 4, 2]
CHUNK = 8192  # gather indices per dma_gather instruction


def layer_specs(out_sizes, fanins, tab0):
    specs = []
    prev = tab0
    for n_out, f in zip(out_sizes, fanins):
        specs.append({"f": f, "n_in": prev, "n_out": n_out, "n_edges": n_out * f})
        prev = n_out
    return specs


def reorder_wrap(ptrs, f, n_out):
    """Permute edge pointers into dma_gather order and wrap into the int16
    [128, n_edges//16] SBUF layout (position j -> [j%16, j//16], replicated
    across the 8 gpsimd cores' 16-partition groups)."""
    C = n_out // P
    n_edges = n_out * f
    j = np.arange(n_edges)
    p = j % P
    slot = j // P
    c = slot // f
    k = slot % f
    g = p * C + c
    src = np.asarray(ptrs).astype(np.int64)[g * f + k]
    assert src.max() < 2**15 and src.min() >= 0
    src = src.astype(np.int16)
    return np.ascontiguousarray(np.tile(src.reshape(-1, 16).T, (8, 1)))


def build_nc(n_vars=N_VARS, out_sizes=OUT_SIZES, fanins=FANINS, chunk=CHUNK):
    import concourse.bacc as bacc
    import concourse.mybir as mybir
    import concourse.tile as tile

    f32 = mybir.dt.float32
    i16 = mybir.dt.int16
    Alu = mybir.AluOpType
    Act = mybir.ActivationFunctionType

    tab0 = 2 * n_vars + 2
    specs = layer_specs(out_sizes, fanins, tab0)
    S_ENC = n_vars // P  # encode slots per partition

    nc = bacc.Bacc("TRN2", target_bir_lowering=False, debug=False)
    x = nc.dram_tensor("x", [P, S_ENC * B], f32, kind="ExternalInput")
    idx_in = [
        nc.dram_tensor(f"idx{l}", [P, s["n_edges"] // 16], i16, kind="ExternalInput")
        for l, s in enumerate(specs)
    ]
    out = nc.dram_tensor("out", [out_sizes[-1], B], f32, kind="ExternalOutput")

    with tile.TileContext(nc) as tc:
        with (
            tc.tile_pool(name="dram", bufs=1, space="DRAM") as dpool,
            tc.tile_pool(name="sb", bufs=4) as gp,
            tc.tile_pool(name="hb", bufs=3) as hp,
            tc.tile_pool(name="tmp", bufs=2) as tp,
            tc.tile_pool(name="ix", bufs=1) as ixp,
        ):
            tables = [
                dpool.tile([s["n_in"], B], f32, name=f"t{l}", tag=f"t{l}")
                for l, s in enumerate(specs)
            ]

            # --- index list loads ---
            ix_t = []
            for l, s in enumerate(specs):
                t = ixp.tile([P, s["n_edges"] // 16], i16, tag=f"ix{l}")
                nc.sync.dma_start(t[:], idx_in[l][:])
                ix_t.append(t)

            # --- encode: pos rows at 2+2i, neg rows at 3+2i, zeros at row 1.
            # Partition p computes vars p*S_ENC .. p*S_ENC+S_ENC-1 so the
            # interleaved pos/neg store is one contiguous run per partition.
            iv = gp.tile([P, S_ENC, 2, B], f32, tag="g")
            nc.sync.dma_start(
                iv[:][:, :, 0, :], x[:].rearrange("p (s b) -> p s b", b=B)
            )
            et = hp.tile([P, S_ENC, B], f32, tag="h")
            nc.scalar.activation(et[:], iv[:][:, :, 0, :], Act.Exp)
            nc.scalar.activation(iv[:][:, :, 1, :], et[:], Act.Ln, scale=-1.0, bias=1.0)
            nc.sync.dma_start(
                tables[0][:][2:, :].rearrange("(p s k) b -> p (s k b)", p=P, k=2),
                iv[:].rearrange("p s k b -> p (s k b)"),
            )
            # rows 0 (-inf in the reference, never gathered) and 1 (zeros)
            z = ixp.tile([2, B], f32, tag="z")
            nc.vector.memset(z[:], 0.0)
            nc.sync.dma_start(tables[0][:][0:2, :], z[:])

            # --- gather + segment-reduce layers ---
            for l, s in enumerate(specs):
                f, n_out, n_edges = s["f"], s["n_out"], s["n_edges"]
                C = n_out // P
                ch = min(chunk if f == 4 else chunk // 2, n_edges)
                assert n_edges % ch == 0
                S = ch // P  # slots per chunk
                Csub = S // f  # groups per partition per chunk
                src_ap = tables[l][:]
                dst_full = (tables[l + 1][:] if l + 1 < len(specs) else out[:]).rearrange(
                    "(p C) b -> p C b", p=P
                )
                for ci in range(n_edges // ch):
                    g = gp.tile([P, S, B], f32, tag="g")
                    nc.gpsimd.dma_gather(
                        g[:],
                        src_ap,
                        ix_t[l][:, ci * (ch // 16) : (ci + 1) * (ch // 16)],
                        ch,
                        ch,
                        B,
                        single_packet=False,
                    )
                    v = g[:].rearrange("p (c k) b -> p c k b", k=f)
                    h = hp.tile([P, Csub, B], f32, tag="h")
                    if f == 4:
                        s01 = tp.tile([P, Csub, B], f32, tag="m")
                        s23 = tp.tile([P, Csub, B], f32, tag="n")
                        nc.vector.tensor_add(s01[:], v[:, :, 0, :], v[:, :, 1, :])
                        nc.vector.tensor_add(s23[:], v[:, :, 2, :], v[:, :, 3, :])
                        nc.vector.tensor_add(h[:], s01[:], s23[:])
                    else:
                        # logsumexp(a,b) = max + ln(1 + exp(min - max))
                        m = tp.tile([P, Csub, B], f32, tag="m")
                        mn = tp.tile([P, Csub, B], f32, tag="n")
                        d = tp.tile([P, Csub, B], f32, tag="d")
                        sp = tp.tile([P, Csub, B], f32, tag="sp")
                        nc.vector.tensor_tensor(
                            m[:], v[:, :, 0, :], v[:, :, 1, :], op=Alu.max
                        )
                        nc.vector.tensor_tensor(
                            mn[:], v[:, :, 0, :], v[:, :, 1, :], op=Alu.min
                        )
                        nc.vector.tensor_tensor(d[:], mn[:], m[:], op=Alu.subtract)
                        nc.scalar.activation(d[:], d[:], Act.Exp)
                        nc.scalar.activation(sp[:], d[:], Act.Ln, bias=1.0)
                        nc.vector.tensor_add(h[:], m[:], sp[:])
                    nc.sync.dma_start(
                        dst_full[:, ci * Csub : (ci + 1) * Csub, :], h[:]
                    )
    nc.compile()
    return nc


def host_prep(x, ptrs_list, seg_list, n_vars=N_VARS, out_sizes=OUT_SIZES, fanins=FANINS):
    """Host-side sharding + index preprocessing. Returns per-core input maps."""
    x = np.asarray(x, dtype=np.float32)
    specs = layer_specs(out_sizes, fanins, 2 * n_vars + 2)
    idx_maps = {}
    for l, s in enumerate(specs):
        seg = np.asarray(seg_list[l]).astype(np.int64)
        expected = np.repeat(np.arange(s["n_out"], dtype=np.int64), s["f"])
        assert np.array_equal(seg, expected), f"layer {l}: non-uniform segments"
        idx_maps[f"idx{l}"] = reorder_wrap(ptrs_list[l], s["f"], s["n_out"])

    batch = x.shape[1]
    bpc = batch // NCORES
    in_maps = []
    for i in range(NCORES):
        xs = x[:, i * bpc : (i + 1) * bpc]
        # partition p holds vars p*S_ENC .. p*S_ENC+S_ENC-1 (natural order)
        xv = np.ascontiguousarray(xs).reshape(P, -1)
        in_maps.append({"x": xv, **idx_maps})
    return in_maps


_CACHE = {}


def _get_nc():
    if "nc" not in _CACHE:
        _CACHE["nc"] = build_nc()
    return _CACHE["nc"]


def kernel(x, ptrs0, seg0, ptrs1, seg1, ptrs2, seg2, ptrs3, seg3):
    from concourse.bass_utils import run_bass_kernel_spmd

    nc = _get_nc()
    in_maps = host_prep(
        x, [ptrs0, ptrs1, ptrs2, ptrs3], [seg0, seg1, seg2, seg3]
    )
    res = run_bass_kernel_spmd(nc, in_maps, core_ids=list(range(NCORES)))
    outs = [r["out"] for r in res.results]
    return np.concatenate(outs, axis=1)
